# revision 16
# baseline (speedup 1.0000x reference)
"""Trainium2 Bass kernel for the 3-layer sparse-attention model (nn_BDH).

Self-contained: kernel(**inputs) takes the FULL inputs (as produced by
setup_inputs()) and returns the FULL [B, T, OUT] logits, distributing work
over 8 NeuronCores as (batch x head-pair): core c handles batch c//4 and
heads {2*(c%4), 2*(c%4)+1}. Per layer, the per-head decoder partials are
summed with a grouped AllReduce within each batch's 4-core group; the
AllReduce is split into two T-halves so the first half's collective
overlaps the tail of the attention compute and the second half's overlaps
the next stage's compute.

Numerics: all matmuls run in fp32r (TF32-like, full PE rate at free-dim
>= 256); end-to-end scale-relative absmax error vs the fp32 reference is
~5e-4 (validated in emulation and on hardware).

Layout/algebra tricks (validated host-side):
- The n-axis of encoder/encoder_v/decoder is de-interleaved host-side, so
  RoPE becomes a half-split rotation with [128, T] cos/sin tables.
- Scores are symmetric (S = QR @ QR^T); the strict-causal mask becomes a
  strict-UPPER mask on S in [s, t] layout, which is exactly the lhsT the
  yKV matmul wants -- the attention inner loop is transpose-free.
- yKV is produced transposed ([d, t]); its LayerNorm is folded away: the
  mean correction enters the encoder_v matmul as a K=1 rank-1 update with
  host-precomputed -sum_d(encoder_v), and 1/sigma is folded into the
  decoder-output accumulation as a per-partition scalar.
"""
import math
import os

import numpy as np

_BUILT = None
LAST_RESULTS = None  # BassKernelResults of the most recent run (for test.py)

B, T, D, NH, MULT, NL = 2, 2048, 512, 8, 4, 3
N = D * MULT // NH          # 256 per-head latent dim
NHALF = N // 2
OUT = 512
EPS = 1e-5
DC = D // 128               # 4 d-chunks
TB = T // 128               # 16 t-blocks
CHUNK = 256                 # S/yKV/Zy chunk width (>=256 keeps fp32r rate)
NCH = T // CHUNK
ZCH = 512                   # Zq/rope chunk width
NZ = T // ZCH
TH = T // 2                 # half length (AllReduce pipelining granularity)


def round_fp32r(x):
    b = np.ascontiguousarray(x, dtype=np.float32).view(np.uint32).astype(np.uint64)
    b = b + 0x7FF + ((b >> np.uint64(12)) & np.uint64(1))
    return (b & np.uint64(0xFFFFF000)).astype(np.uint32).view(np.float32)


def build():
    from contextlib import ExitStack

    import concourse.bacc as bacc
    import concourse.tile as tile
    import concourse.tile_utils as tile_utils
    from concourse import mybir

    if getattr(tile_utils, "max_sbuf_usage", 0) < 208 * 1024:
        tile_utils.max_sbuf_usage = 208 * 1024

    f32 = mybir.dt.float32
    f32r = mybir.dt.float32r
    bf16 = mybir.dt.bfloat16
    AF = mybir.ActivationFunctionType
    ALU = mybir.AluOpType

    nc = bacc.Bacc("TRN2", target_bir_lowering=False, debug=False, num_devices=8)

    xT_p = nc.declare_dram_parameter("xT", [DC, 128, T], f32r, isOutput=False)
    w_in_p = nc.declare_dram_parameter("w_in", [DC, 128, D], f32r, isOutput=False)
    b_in_p = nc.declare_dram_parameter("b_in_row", [1, D], f32r, isOutput=False)
    enc_p = nc.declare_dram_parameter("enc", [2, DC, 128, N], f32r, isOutput=False)
    encv_p = nc.declare_dram_parameter("encv", [2, DC, 128, N], f32r,
                                       isOutput=False)
    dec_p = nc.declare_dram_parameter("dec", [2, 2, NHALF, D], f32r,
                                      isOutput=False)
    esum_p = nc.declare_dram_parameter("esum", [2, 1, N], f32r, isOutput=False)
    cos_p = nc.declare_dram_parameter("cosT", [NHALF, T], f32r, isOutput=False)
    sin_p = nc.declare_dram_parameter("sinT", [NHALF, T], f32r, isOutput=False)
    mask_p = nc.declare_dram_parameter("maskT", [128, 128], f32, isOutput=False)
    ident_p = nc.declare_dram_parameter("ident", [128, 128], f32r, isOutput=False)
    onesd_p = nc.declare_dram_parameter("onesd", [128, 1], f32r, isOutput=False)
    ones128_p = nc.declare_dram_parameter("ones128", [1, 128], f32r,
                                          isOutput=False)
    onesrow_p = nc.declare_dram_parameter("onesrow", [1, OUT], f32r,
                                          isOutput=False)
    hw_p = nc.declare_dram_parameter("head_w", [DC, 128, OUT], f32r,
                                     isOutput=False)
    hb_p = nc.declare_dram_parameter("head_b_row", [1, OUT], f32r,
                                     isOutput=False)
    out_p = nc.declare_dram_parameter("logitsT", [OUT, T], f32, isOutput=True)

    with tile.TileContext(nc) as tc, ExitStack() as ctx:
        const = ctx.enter_context(tc.tile_pool(name="const", bufs=1))
        state = ctx.enter_context(tc.tile_pool(name="state", bufs=1))
        wstream = ctx.enter_context(tc.tile_pool(name="wstream", bufs=1))
        work = ctx.enter_context(tc.tile_pool(name="work", bufs=1))
        spool = ctx.enter_context(tc.tile_pool(name="spool", bufs=1))
        small = ctx.enter_context(tc.tile_pool(name="small", bufs=2))
        psum = ctx.enter_context(tc.tile_pool(name="psum", bufs=1, space="PSUM"))
        dram = ctx.enter_context(tc.tile_pool(name="dram", bufs=1, space="DRAM"))

        # ---------------- constants ----------------
        cosT = const.tile([NHALF, T], f32r)
        sinT = const.tile([NHALF, T], f32r)
        maskT = const.tile([128, 128], f32)
        ident = const.tile([128, 128], f32r)
        onesd = const.tile([128, 1], f32r)
        ones128 = const.tile([1, 128], f32r)

        b_in_row = const.tile([1, D], f32r)
        hb_row = const.tile([1, OUT], f32r)
        esum_sb = [const.tile([1, N], f32r, name=f"esum{h}") for h in range(2)]
        epsc = const.tile([128, 1], f32)
        nc.sync.dma_start(cosT[:], cos_p[:])
        nc.sync.dma_start(sinT[:], sin_p[:])
        nc.sync.dma_start(maskT[:], mask_p[:])
        nc.sync.dma_start(ident[:], ident_p[:])
        nc.sync.dma_start(onesd[:], onesd_p[:])
        nc.sync.dma_start(ones128[:], ones128_p[:])
        nc.sync.dma_start(b_in_row[:], b_in_p[:])
        nc.sync.dma_start(hb_row[:], hb_p[:])
        for h in range(2):
            nc.sync.dma_start(esum_sb[h][:], esum_p[h])
        nc.vector.memset(epsc[:], EPS)

        # ------------- persistent state (T-half granular) -------------
        xs = [state.tile([128, D], f32r, name=f"xs{tb}") for tb in range(TB)]
        xsT = [[state.tile([128, TH], f32r, name=f"xsT{dc}_{hf}")
                for hf in range(2)] for dc in range(DC)]
        yacc = [state.tile([128, D], f32, name=f"yacc{tb}") for tb in range(TB)]
        Q = [[state.tile([128, TH], f32r, name=f"Qh{nt}_{hf}")
              for hf in range(2)] for nt in range(2)]
        QR = [[state.tile([128, TH], f32r, name=f"QRh{nt}_{hf}")
               for hf in range(2)] for nt in range(2)]

        def half_ap(tiles2, c0, c1):
            """AP for columns [c0:c1) of a T-range stored as two TH tiles.
            The range must not cross the half boundary."""
            hf = c0 // TH
            assert (c1 - 1) // TH == hf, (c0, c1)
            return tiles2[hf][:, c0 - hf * TH:c1 - hf * TH]

        ar_in = [dram.tile([TH, D], f32, name=f"ar_in{hf}") for hf in range(2)]
        ar_out = [dram.tile([TH, D], f32, name=f"ar_out{hf}") for hf in range(2)]
        mu_b = dram.tile([1, CHUNK], f32, name="mu_b", tag="mu_b", bufs=2)
        msq_b = dram.tile([1, CHUNK], f32, name="msq_b", tag="msq_b", bufs=2)
        # (row tiles above are DRAM; cheap)

        def ln_tile(dst_ap, src_ap):
            bn6 = small.tile([128, 6], f32, name="bn6", tag="bn6")
            bn2 = small.tile([128, 2], f32, name="bn2", tag="bn2")
            sd = small.tile([128, 1], f32, name="sd", tag="sd")
            rs = small.tile([128, 1], f32, name="rs", tag="rs")
            nc.vector.bn_stats(bn6[:], src_ap)
            nc.vector.bn_aggr(bn2[:], bn6[:])
            nc.scalar.activation(sd[:], bn2[:, 1:2], AF.Sqrt, bias=epsc[:])
            nc.vector.reciprocal(rs[:], sd[:])
            nc.vector.tensor_scalar(dst_ap, src_ap, bn2[:, 0:1], rs[:],
                                    ALU.subtract, ALU.mult)

        def transpose_half(hf):
            for dc in range(DC):
                for tbl in range(TB // 2):
                    tb = hf * (TB // 2) + tbl
                    pt = psum.tile([128, 128], f32r, name="ptr", tag="sz",
                                   bufs=3)
                    nc.tensor.transpose(pt[:],
                                        xs[tb][:, dc * 128:(dc + 1) * 128],
                                        ident[:])
                    dst = xsT[dc][hf][:, tbl * 128:(tbl + 1) * 128]
                    if (dc + tbl) % 2 == 0:
                        nc.vector.tensor_copy(dst, pt[:])
                    else:
                        nc.scalar.activation(dst, pt[:], AF.Copy)

        # =========================================================
        # input projection: xs0 = ln(x @ w_in + b_in)   [T, D]
        # =========================================================
        w_in_sb = [wstream.tile([128, D], f32r, name=f"win{dc}",
                                tag=f"w{dc}") for dc in range(DC)]
        for dc in range(DC):
            nc.sync.dma_start(w_in_sb[dc][:], w_in_p[dc])
        for tb in range(TB):
            xt_sb = [spool.tile([128, 128], f32r, name=f"xt{tb}_{dc}",
                                tag=f"s{dc}") for dc in range(DC)]
            for dc in range(DC):
                nc.sync.dma_start(xt_sb[dc][:],
                                  xT_p[dc, :, tb * 128:(tb + 1) * 128])
            pz = psum.tile([128, D], f32, name="pz", tag="ym", bufs=2)
            for dc in range(DC):
                nc.tensor.matmul(pz[:], xt_sb[dc][:],
                                 w_in_sb[dc][:], start=(dc == 0), stop=False)
            nc.tensor.matmul(pz[:], ones128[:], b_in_row[:], start=False,
                             stop=True)
            ln_tile(xs[tb][:], pz[:])
        for hf in range(2):
            transpose_half(hf)

        # =========================================================
        # layers
        # =========================================================
        for layer in range(NL):
            for hi in range(2):
                enc_sb = [wstream.tile([128, N], f32r, name=f"enc{layer}{hi}{dc}",
                                       tag=f"w{dc}") for dc in range(DC)]
                for dc in range(DC):
                    nc.sync.dma_start(enc_sb[dc][:], enc_p[hi, dc])

                # ---- Zq + rope -> Q, QR ----
                for ci in range(NZ):
                    c0, c1 = ci * ZCH, (ci + 1) * ZCH
                    zq = []
                    for nt in range(2):
                        pq = psum.tile([128, ZCH], f32, name=f"zq{nt}",
                                       tag=("ym" if nt == 0 else "sz"),
                                       bufs=(2 if nt == 0 else 3))
                        for dc in range(DC):
                            nc.tensor.matmul(
                                pq[:], enc_sb[dc][:, nt * 128:(nt + 1) * 128],
                                half_ap(xsT[dc], c0, c1),
                                start=(dc == 0), stop=(dc == DC - 1))
                        zq.append(pq)
                    for nt in range(2):
                        nc.scalar.activation(half_ap(Q[nt], c0, c1), zq[nt][:],
                                             AF.Relu)
                    tmp = work.tile([128, ZCH], f32r, name="ropetmp",
                                    tag="rtmp", bufs=1)
                    tmp2 = work.tile([128, ZCH], f32r, name="ropetmp2",
                                     tag="rtmp2", bufs=1)
                    nc.vector.scalar_tensor_tensor(
                        tmp[:], zq[1][:], 0.0, sinT[:, c0:c1], ALU.max, ALU.mult)
                    nc.vector.scalar_tensor_tensor(
                        tmp2[:], zq[0][:], 0.0, cosT[:, c0:c1], ALU.max, ALU.mult)
                    nc.gpsimd.tensor_tensor(half_ap(QR[0], c0, c1), tmp2[:],
                                            tmp[:], ALU.subtract)
                    nc.vector.scalar_tensor_tensor(
                        tmp[:], zq[0][:], 0.0, sinT[:, c0:c1], ALU.max, ALU.mult)
                    nc.vector.scalar_tensor_tensor(
                        tmp2[:], zq[1][:], 0.0, cosT[:, c0:c1], ALU.max, ALU.mult)
                    nc.gpsimd.tensor_tensor(half_ap(QR[1], c0, c1), tmp2[:],
                                            tmp[:], ALU.add)

                encv_sb = [wstream.tile([128, N], f32r, name=f"env{layer}{hi}{dc}",
                                        tag=f"w{dc}") for dc in range(DC)]
                for dc in range(DC):
                    nc.sync.dma_start(encv_sb[dc][:], encv_p[hi, dc])
                dec_sb = [wstream.tile([NHALF, D], f32r, name=f"dec{layer}{hi}{nt}",
                                       tag=f"d{nt}") for nt in range(2)]
                for nt in range(2):
                    nc.sync.dma_start(dec_sb[nt][:], dec_p[hi, nt])

                mu_col = work.tile([128, TB], f32, name="mu_col", tag="mu_col")
                msq_col = work.tile([128, TB], f32, name="msq_col", tag="msq_col")
                rsig_col = work.tile([128, TB], f32, name="rsig_col",
                                     tag="rsig_col")

                for tcid in range(NCH):
                    t0, t1 = tcid * CHUNK, (tcid + 1) * CHUNK
                    nsb = t1 // 128
                    # ---- masked score tiles, [s, t] layout, f32r ----
                    s_tiles = []
                    for j in range(nsb):
                        s0 = j * 128
                        cc0 = max(t0, s0)
                        w = t1 - cc0
                        ps = psum.tile([128, CHUNK], f32, name="ps_s",
                                       tag="sz", bufs=3)
                        for nt in range(2):
                            nc.tensor.matmul(ps[:, 0:w],
                                             half_ap(QR[nt], s0, s0 + 128),
                                             half_ap(QR[nt], cc0, t1),
                                             start=(nt == 0), stop=(nt == 1))
                        st = spool.tile([128, CHUNK], f32r, name=f"s{j}",
                                        tag=f"s{j}")
                        if s0 >= t0:
                            dcols = min(s0 + 128, t1) - cc0
                            nc.vector.tensor_tensor(st[:, 0:dcols],
                                                    ps[:, 0:dcols],
                                                    maskT[:, 0:dcols], ALU.mult)
                            if w > dcols:
                                nc.scalar.activation(st[:, dcols:w],
                                                     ps[:, dcols:w], AF.Copy)
                        elif j % 3 == 0:
                            nc.vector.tensor_copy(st[:, 0:w], ps[:, 0:w])
                        else:
                            nc.scalar.activation(st[:, 0:w], ps[:, 0:w], AF.Copy)
                        s_tiles.append((st, cc0 - t0, w))

                    # ---- yKV^T [d, chunk], one d-tile at a time ----
                    ykv_sb = [work.tile([128, CHUNK], f32r, name=f"ykvsb{dt}",
                                        tag=f"ykvsb{dt}") for dt in range(DC)]
                    sq_sb = [work.tile([128, CHUNK], f32r, name=f"sqsb{dt}",
                                       tag=f"sqsb{dt}") for dt in range(DC)]
                    for dt in range(DC):
                        pykv = psum.tile([128, CHUNK], f32, name="pykv",
                                         tag="ykv", bufs=2)
                        for j in range(nsb):
                            st, off, w = s_tiles[j]
                            nc.tensor.matmul(
                                pykv[:, off:off + w],
                                xs[j][:, dt * 128:(dt + 1) * 128],
                                st[:, 0:w],
                                start=(j == 0), stop=(j == nsb - 1))
                        nc.scalar.activation(ykv_sb[dt][:], pykv[:], AF.Copy)
                        nc.gpsimd.tensor_tensor(sq_sb[dt][:], ykv_sb[dt][:],
                                                ykv_sb[dt][:], ALU.mult)

                    # ---- stats ----
                    pmu = psum.tile([1, CHUNK], f32, name="pmu", tag="st")
                    for dt in range(DC):
                        nc.tensor.matmul(pmu[:], onesd[:], ykv_sb[dt][:],
                                         start=(dt == 0), stop=(dt == DC - 1))
                    mu_row = work.tile([1, CHUNK], f32r, name="mu_row",
                                       tag="mu_row", bufs=1)
                    nc.vector.tensor_copy(mu_row[:], pmu[:])
                    nc.sync.dma_start(mu_b[:], mu_row[:].bitcast(f32))
                    nc.sync.dma_start(
                        mu_col[:, 2 * tcid:2 * tcid + 2],
                        mu_b[:].rearrange("one (c p) -> (one p) c", p=128))
                    pmsq = psum.tile([1, CHUNK], f32, name="pmsq", tag="st")
                    for dt in range(DC):
                        nc.tensor.matmul(pmsq[:], onesd[:], sq_sb[dt][:],
                                         start=(dt == 0), stop=(dt == DC - 1))
                    msq_row = work.tile([1, CHUNK], f32, name="msq_row",
                                        tag="msq_row", bufs=1)
                    nc.vector.tensor_copy(msq_row[:], pmsq[:])
                    nc.sync.dma_start(msq_b[:], msq_row[:])
                    nc.sync.dma_start(
                        msq_col[:, 2 * tcid:2 * tcid + 2],
                        msq_b[:].rearrange("one (c p) -> (one p) c", p=128))
                    c2 = slice(2 * tcid, 2 * tcid + 2)
                    tmu = small.tile([128, 2], f32, name="tmu", tag="tmu")
                    tsd = small.tile([128, 2], f32, name="tsd", tag="tsd")
                    nc.vector.tensor_tensor(tmu[:], mu_col[:, c2],
                                            mu_col[:, c2], ALU.mult)
                    nc.vector.tensor_tensor(tmu[:], msq_col[:, c2], tmu[:],
                                            ALU.subtract)
                    nc.scalar.activation(tsd[:], tmu[:], AF.Sqrt, bias=epsc[:])
                    nc.vector.reciprocal(rsig_col[:, c2], tsd[:])

                    # ---- Zy (+ K=1 mean correction) -> xy ----
                    xy_sb = [work.tile([128, CHUNK], f32r, name=f"xy{nt}",
                                       tag=f"xy{nt}") for nt in range(2)]
                    for nt in range(2):
                        pzy = psum.tile([128, CHUNK], f32, name="pzy",
                                        tag="sz", bufs=3)
                        for dc in range(DC):
                            nc.tensor.matmul(
                                pzy[:], encv_sb[dc][:, nt * 128:(nt + 1) * 128],
                                ykv_sb[dc][:], start=(dc == 0), stop=False)
                        nc.tensor.matmul(
                            pzy[:], esum_sb[hi][:, nt * 128:(nt + 1) * 128],
                            mu_row[:], start=False, stop=True)
                        nc.vector.scalar_tensor_tensor(
                            xy_sb[nt][:], pzy[:], 0.0,
                            half_ap(Q[nt], t0, t1), ALU.max, ALU.mult)

                    # ---- yMLP partial (rsig folded into evacuation) ----
                    for bi in range(CHUNK // 128):
                        tb = 2 * tcid + bi
                        pym = psum.tile([128, D], f32, name="pym", tag="ym",
                                        bufs=2)
                        for nt in range(2):
                            nc.tensor.matmul(
                                pym[:], xy_sb[nt][:, bi * 128:(bi + 1) * 128],
                                dec_sb[nt][:], start=(nt == 0), stop=(nt == 1))
                        if hi == 0:
                            nc.scalar.activation(yacc[tb][:], pym[:], AF.Copy,
                                                 scale=rsig_col[:, tb:tb + 1])
                        else:
                            nc.vector.scalar_tensor_tensor(
                                yacc[tb][:], pym[:], rsig_col[:, tb:tb + 1],
                                yacc[tb][:], ALU.mult, ALU.add)

                    # head 1 finishing a half kicks off that half's AllReduce
                    if hi == 1 and t1 in (TH, T):
                        hf = t1 // TH - 1
                        for tbl in range(TB // 2):
                            tb = hf * (TB // 2) + tbl
                            nc.sync.dma_start(
                                ar_in[hf][tbl * 128:(tbl + 1) * 128, :],
                                yacc[tb][:])
                        nc.gpsimd.collective_compute(
                            "AllReduce", ALU.add,
                            replica_groups=[[0, 1, 2, 3], [4, 5, 6, 7]],
                            ins=[ar_in[hf].opt()], outs=[ar_out[hf].opt()],
                        )

            # ---- xs = ln(x_res + ln(ymlp)), in place; then transpose ----
            for hf in range(2):
                for tbl in range(TB // 2):
                    tb = hf * (TB // 2) + tbl
                    yt = work.tile([128, D], f32, name="ln_in", tag="ln_in",
                                   bufs=2)
                    nc.sync.dma_start(yt[:],
                                      ar_out[hf][tbl * 128:(tbl + 1) * 128, :])
                    n1 = work.tile([128, D], f32, name="ln_n1", tag="ln_n1",
                                   bufs=1)
                    ln_tile(n1[:], yt[:])
                    u = work.tile([128, D], f32, name="ln_u", tag="ln_in",
                                  bufs=2)
                    nc.gpsimd.tensor_tensor(u[:], n1[:], xs[tb][:], ALU.add)
                    ln_tile(xs[tb][:], u[:])
                transpose_half(hf)

        # =========================================================
        # head: logitsT [OUT, T] = head_w^T @ xs^T + head_b
        # =========================================================
        hw_sb = [wstream.tile([128, OUT], f32r, name=f"hw{dc}", tag=f"w{dc}")
                 for dc in range(DC)]
        for dc in range(DC):
            nc.sync.dma_start(hw_sb[dc][:], hw_p[dc])
        for ot in range(OUT // 128):
            for ci in range(NZ):
                c0, c1 = ci * ZCH, (ci + 1) * ZCH
                ph = psum.tile([128, ZCH], f32, name="ph", tag="ym", bufs=2)
                for dc in range(DC):
                    nc.tensor.matmul(ph[:],
                                     hw_sb[dc][:, ot * 128:(ot + 1) * 128],
                                     half_ap(xsT[dc], c0, c1),
                                     start=(dc == 0), stop=False)
                for qi in range(ZCH // 128):
                    nc.tensor.matmul(
                        ph[:, qi * 128:(qi + 1) * 128],
                        hb_row[:, ot * 128:(ot + 1) * 128],
                        ones128[:], start=False, stop=(qi == ZCH // 128 - 1),
                        skip_group_check=True)
                ot_sb = work.tile([128, ZCH], f32, name="ot_sb", tag="ln_in",
                                  bufs=2)
                nc.scalar.activation(ot_sb[:], ph[:], AF.Copy)
                nc.sync.dma_start(out_p[ot * 128:(ot + 1) * 128, c0:c1],
                                  ot_sb[:])
    nc.compile()
    return nc


def _host_prep(inputs):
    x = np.asarray(inputs["x"], np.float32)
    w_in = np.asarray(inputs["w_in"], np.float32)
    b_in = np.asarray(inputs["b_in"], np.float32)
    encoder = np.asarray(inputs["encoder"], np.float32)
    encoder_v = np.asarray(inputs["encoder_v"], np.float32)
    decoder = np.asarray(inputs["decoder"], np.float32)
    head_w = np.asarray(inputs["head_w"], np.float32)
    head_b = np.asarray(inputs["head_b"], np.float32)

    perm = np.concatenate([np.arange(0, N, 2), np.arange(1, N, 2)])
    dec3 = decoder.reshape(NH, N, D)
    encp = round_fp32r(encoder[:, :, perm])
    encvp = round_fp32r(encoder_v[:, :, perm])
    decp = round_fp32r(dec3[:, perm, :])
    esum_neg = round_fp32r(-encvp.sum(axis=1, dtype=np.float64).astype(np.float32))
    theta = 2.0 ** 16
    q = np.floor(np.arange(N) / 2.0) * 2.0
    freqs = (1.0 / theta ** (q / N) / (2.0 * math.pi)).astype(np.float32)
    fr = freqs[perm][:NHALF].astype(np.float64)
    ph = (np.arange(T, dtype=np.float64)[None, :] * fr[:, None]) % 1.0
    cosT = round_fp32r(np.cos(2 * math.pi * ph).astype(np.float32))
    sinT = round_fp32r(np.sin(2 * math.pi * ph).astype(np.float32))
    maskT = np.triu(np.ones((128, 128), np.float32), 1)
    ident = round_fp32r(np.eye(128, dtype=np.float32))
    onesd = round_fp32r(np.full((128, 1), 1.0 / D, np.float32))
    ones128 = round_fp32r(np.ones((1, 128), np.float32))
    onesrow = round_fp32r(np.ones((1, OUT), np.float32))
    w_inr = round_fp32r(w_in).reshape(DC, 128, D)
    head_wr = round_fp32r(head_w).reshape(DC, 128, OUT)
    b_in_row = round_fp32r(b_in).reshape(1, D)
    hb_row = round_fp32r(head_b).reshape(1, OUT)

    in_maps = []
    for c in range(8):
        b = c // 4
        hs = [2 * (c % 4), 2 * (c % 4) + 1]
        in_maps.append({
            "xT": round_fp32r(x[b].T).reshape(DC, 128, T).copy(),
            "w_in": w_inr,
            "b_in_row": b_in_row,
            "enc": encp[hs].reshape(2, DC, 128, N).copy(),
            "encv": encvp[hs].reshape(2, DC, 128, N).copy(),
            "dec": decp[hs].reshape(2, 2, NHALF, D).copy(),
            "esum": esum_neg[hs].reshape(2, 1, N).copy(),
            "cosT": cosT,
            "sinT": sinT,
            "maskT": maskT,
            "ident": ident,
            "onesd": onesd,
            "ones128": ones128,
            "onesrow": onesrow,
            "head_w": head_wr,
            "head_b_row": hb_row,
        })
    return in_maps


def kernel(**inputs):
    from concourse.bass_utils import run_bass_kernel_spmd
    global _BUILT, LAST_RESULTS
    if _BUILT is None:
        _BUILT = build()
    in_maps = _host_prep(inputs)
    trace = os.environ.get("KERNEL_TRACE", "0") == "1"
    r = run_bass_kernel_spmd(_BUILT, in_maps, list(range(8)), trace=trace)
    LAST_RESULTS = r
    out = np.empty((B, T, OUT), np.float32)
    for b in range(B):
        out[b] = r.results[4 * b]["logitsT"].T
    return out
